# revision 1
# baseline (speedup 1.0000x reference)
"""Trainium2 Bass kernel for a 3-layer GCN (nn_GCN_37383395344580).

Strategy (8 NeuronCores, one SPMD program):
  - Nodes are dealt round-robin by in-degree across 8 cores x 98 windows of
    128 dst slots (balances the SPMD max-over-cores edge padding); each core
    aggregates its windows' incoming edges (incl. self loops).
  - norm factorizes: norm(s,d) = dinv[s]*dinv[d], so messages are rows of a
    replicated bf16 "table" T = dinv * (h @ W) and aggregated sums are
    rescaled by dinv[d]: zero per-edge vector work.
  - Per layer: per-window GEMM + row scale feed 4 quarter-shard AllGathers
    (pipelined with the previous layer's gather passes); 4 gather passes of
    dma_gather (int16 indices address one quarter table, 256B rows); one
    batched is_equal builds 64 one-hot selection matrices per DVE op; window
    matmuls (edges = contraction dim) accumulate [128 dst x 64] in PSUM;
    window close-out chains epilogue -> next-layer GEMM -> quarter AllGather.
  - Final: one-hot graph-id matmuls pool per-graph sums, AllReduce across
    cores, scale by host-computed 1/max(cnt,1).

The per-edge schedule (window/quarter run lengths, gather calls, close-out
points) is JIT-specialized to the actual graph inside kernel() but identical
across cores (SPMD): run lengths are max-reduced over cores and each core
pads its own index streams (pad edges gather row 0 with dstloc=-1, zeroing
their one-hot row).

Hardware notes learned on TRN2:
  - dma_gather/dma_scatter_add need gpsimd.load_library(library_config.mlp).
  - single_packet=True hangs beyond ~1024 indices/call; use
    single_packet=False for large calls.
  - The Q7 SWDGE descriptor generation (~5.6ns/row) is the kernel's floor;
    everything else (DVE one-hots, PE matmuls, collectives, HBM traffic) is
    arranged to hide behind it.
"""

import os
import sys
from dataclasses import dataclass

import numpy as np

for _p in ("/opt/trn_rl_repo",):
    if _p not in sys.path and os.path.isdir(_p):
        sys.path.insert(0, _p)

import concourse.bass as bass
import concourse.bacc as bacc
import concourse.tile as tile
from concourse import library_config, mybir

P = 128  # partitions


@dataclass(frozen=True)
class Cfg:
    N: int = 100000       # nodes
    F: int = 64           # feature width (all layers; layer-3 W padded)
    OUT: int = 32         # final feature width
    G: int = 64           # graphs
    C: int = 8            # cores
    NQ: int = 4           # gather quadrants (int16 index limit)
    GCH: int = 64         # max subchunks (of 128 edges) per dma_gather call
    table_bf16: bool = True  # bf16 gather table (half AllGather bytes, 4x LDW)
    dma_scratch: int = 16384  # SWDGE descriptor carveout bytes/partition
    single_packet: bool = False
    swdge_queues: int = 4
    ship_delay: int = 2   # gather calls between quarter-GEMM done and its AG

    @property
    def NLOC(self):
        assert self.N % self.C == 0
        return self.N // self.C

    @property
    def NT(self):
        return -(-self.NLOC // P)

    @property
    def PAD(self):
        return self.NT * P

    @property
    def TR(self):
        return self.C * self.PAD

    @property
    def QR(self):
        assert self.TR % self.NQ == 0
        return self.TR // self.NQ

    @property
    def TC(self):  # table row width in elements (row stride must be 256B)
        return 2 * self.F if self.table_bf16 else self.F

    @property
    def qtiles(self):
        """Tiles per quarter-shard AllGather (pipelined with the GEMM)."""
        base = [self.NT // self.NQ] * self.NQ
        for i in range(self.NT % self.NQ):
            base[i] += 1
        return base

    @property
    def SDT(self):
        return mybir.dt.bfloat16 if self.table_bf16 else mybir.dt.float32


FULL = Cfg()


# --------------------------------------------------------------------------
# Host-side schedule + per-core stream construction (pure numpy)
# --------------------------------------------------------------------------

def node_placement(dst, cfg: Cfg):
    """Permute nodes across (core, window, lane) slots to balance per-window
    in-degree (cuts SPMD max-over-cores padding). Returns (node_core, node_l)
    where node_l = local index (window*128 + lane)."""
    N, C, NT = cfg.N, cfg.C, cfg.NT
    deg = np.bincount(np.asarray(dst, dtype=np.int64), minlength=N)
    order = np.argsort(-deg, kind="stable")      # high degree first
    NW = C * NT
    rank = np.empty(N, dtype=np.int64)
    rank[order] = np.arange(N)
    wslot = rank % NW                             # round-robin over all windows
    lane = rank // NW
    node_core = wslot // NT
    node_w = wslot % NT
    node_l = node_w * P + lane
    return node_core, node_l


def build_schedule(src, dst, cfg: Cfg):
    """src/dst incl. self loops. Quarter q of a node = which quarter-shard AG
    delivers its table row. Returns (sched, percore_gidx, percore_dstloc,
    node_core, node_l)."""
    N, C, NQ = cfg.N, cfg.C, cfg.NQ
    NT, PADR = cfg.NT, cfg.PAD
    QTILES = cfg.qtiles                 # tiles per quarter, sums to NT
    QB = np.concatenate([[0], np.cumsum(np.array(QTILES) * P)])  # local row bnds

    s = np.asarray(src, dtype=np.int64)
    d = np.asarray(dst, dtype=np.int64)
    node_core, node_l = node_placement(d, cfg)

    l_s = node_l[s]
    q = np.searchsorted(QB, l_s, side="right") - 1
    qsize = np.diff(QB)                       # local rows per quarter
    gidx_val = (node_core[s] * qsize[q] + (l_s - QB[q])).astype(np.int16)

    c = node_core[d]
    dl = node_l[d]
    w = dl // P
    dloc = dl % P

    # Superblock run order: windows grouped by their own quarter; all 4
    # src-quarter passes run back-to-back per superblock, so quarter-B windows
    # finalize (and ship next-layer tables) at ~(B+1)/4 through the layer.
    NR = NQ * NT
    run_q = np.empty(NR, dtype=np.int64)
    run_w = np.empty(NR, dtype=np.int64)
    runpos = np.empty((NQ, NT), dtype=np.int64)
    tile_q = np.searchsorted(QB, np.arange(NT) * P, side="right") - 1
    r = 0
    for B in range(NQ):
        ws = np.nonzero(tile_q == B)[0]
        for qq in range(NQ):
            for w_ in ws:
                run_q[r] = qq
                run_w[r] = w_
                runpos[qq, w_] = r
                r += 1
    assert r == NR

    key = c * NR + runpos[q, w]
    counts = np.bincount(key, minlength=C * NR).reshape(C, NR)
    nsub = -(-counts.max(axis=0) // P)          # [NR] in run order
    sub_base = np.zeros(NR + 1, dtype=np.int64)
    np.cumsum(nsub, out=sub_base[1:])
    TS = int(sub_base[-1])
    SLOTS = TS * P

    r_of_sub = np.searchsorted(sub_base, np.arange(TS), side="right") - 1
    sub_q = run_q[r_of_sub]
    sub_w = run_w[r_of_sub]
    sub_first = np.zeros(TS, dtype=bool)
    sub_last = np.zeros(TS, dtype=bool)
    sub_first[sub_base[:-1][nsub > 0]] = True
    sub_last[sub_base[1:][nsub > 0] - 1] = True
    # final pass per window: its last nonempty run in run order
    final_q = np.zeros(NT, dtype=np.int64)
    for w_i in range(NT):
        rs = runpos[:, w_i]
        nz = rs[nsub[rs] > 0]
        final_q[w_i] = run_q[nz[-1]] if len(nz) else 0

    calls = []   # dicts: q, gs0, n — contiguous same-q subchunk segments
    seg = []
    for rr in range(NR):
        if seg and run_q[rr] != run_q[seg[-1]]:
            lo, hi = int(sub_base[seg[0]]), int(sub_base[seg[-1] + 1])
            gs0 = lo
            while gs0 < hi:
                n = min(cfg.GCH, hi - gs0)
                calls.append(dict(q=int(run_q[seg[0]]), gs0=gs0, n=n))
                gs0 += n
            seg = []
        seg.append(rr)
    if seg:
        lo, hi = int(sub_base[seg[0]]), int(sub_base[seg[-1] + 1])
        gs0 = lo
        while gs0 < hi:
            n = min(cfg.GCH, hi - gs0)
            calls.append(dict(q=int(run_q[seg[0]]), gs0=gs0, n=n))
            gs0 += n
    calls = [cl for cl in calls if cl["n"] > 0]
    GCOLS = SLOTS // 16

    order = np.argsort(key, kind="stable")
    key_sorted = key[order]
    run_first_idx = np.searchsorted(key_sorted, np.arange(C * NQ * NT), side="left")
    edge_order_pos = np.empty(len(s), dtype=np.int64)
    edge_order_pos[order] = np.arange(len(s)) - run_first_idx[key_sorted]

    slot = sub_base[key % (NQ * NT)] * P + edge_order_pos

    percore_gidx = []
    percore_dstloc = []
    for cc in range(C):
        m = c == cc
        gfull = np.zeros(SLOTS, dtype=np.int16)
        dfull = np.full(SLOTS, -1.0, dtype=np.float64)
        gfull[slot[m]] = gidx_val[m]
        dfull[slot[m]] = dloc[m]
        packed = np.ascontiguousarray(
            np.tile(gfull.reshape(GCOLS, 16).T, (8, 1)))
        percore_gidx.append(packed)
        percore_dstloc.append(np.ascontiguousarray(dfull.reshape(TS, P).T))

    sched = dict(TS=TS, GCOLS=GCOLS, calls=calls, sub_q=sub_q, sub_w=sub_w,
                 sub_first=sub_first, sub_last=sub_last, final_q=final_q)
    return sched, percore_gidx, percore_dstloc, node_core, node_l


def host_prep(x, edge_index, batch, W1, b1, W2, b2, W3, b3, cfg: Cfg):
    """Build in_maps (list of dicts per core)."""
    N, F, C, G = cfg.N, cfg.F, cfg.C, cfg.G
    NLOC, NT, PADR = cfg.NLOC, cfg.NT, cfg.PAD

    e0 = np.asarray(edge_index[0], dtype=np.int64)
    e1 = np.asarray(edge_index[1], dtype=np.int64)
    loops = np.arange(N, dtype=np.int64)
    s = np.concatenate([e0, loops])
    d = np.concatenate([e1, loops])

    deg = np.bincount(d, minlength=N).astype(np.float64)
    dinv = (1.0 / np.sqrt(np.maximum(deg, 1.0))).astype(np.float32)

    sched, percore_gidx, percore_dstloc, node_core, node_l = \
        build_schedule(s, d, cfg)

    batch = np.asarray(batch, dtype=np.int64)
    cnts = np.bincount(batch, minlength=G).astype(np.float64)
    invcnt = (1.0 / np.maximum(cnts, 1.0)).astype(np.float32)[:, None]

    W3p = np.zeros((F, F), np.float32)
    W3p[:, :cfg.OUT] = np.asarray(W3, np.float32)
    b3p = np.zeros((F,), np.float32)
    b3p[:cfg.OUT] = np.asarray(b3, np.float32)
    wmat = np.concatenate([np.asarray(W1, np.float32),
                           np.asarray(W2, np.float32), W3p], axis=1)
    bias = np.broadcast_to(
        np.concatenate([np.asarray(b1, np.float32),
                        np.asarray(b2, np.float32), b3p])[None, :], (P, 3 * F)
    ).copy()

    iota_f32 = np.broadcast_to(np.arange(P, dtype=np.float32)[None, :], (P, P)).copy()
    ident = np.eye(P, dtype=np.float32)

    x = np.asarray(x, np.float32)
    in_maps = []
    for cc in range(C):
        m = node_core == cc
        ls = node_l[m]
        xs = np.zeros((PADR, F), np.float32)
        xs[ls] = x[m]
        x_arr = np.ascontiguousarray(
            xs.reshape(NT, P, F).transpose(1, 0, 2).reshape(P, NT * F))

        dv = np.zeros((PADR,), np.float32)
        dv[ls] = dinv[m]
        dinvt = np.ascontiguousarray(dv.reshape(NT, P).T)

        bl = np.full((PADR,), -1.0, np.float32)
        bl[ls] = batch[m].astype(np.float32)
        batchloc = np.ascontiguousarray(bl.reshape(NT, P).T)

        dstloc = percore_dstloc[cc].astype(np.float32)

        iota_sdt = iota_f32
        if cfg.table_bf16:
            import ml_dtypes
            iota_sdt = iota_f32.astype(ml_dtypes.bfloat16)

        in_maps.append({
            "x_arr": x_arr,
            "gidx": percore_gidx[cc],
            "dstloc": dstloc,
            "dinvt": dinvt,
            "batchloc": batchloc,
            "invcnt": invcnt,
            "iota": iota_f32,
            "iota_sdt": np.ascontiguousarray(iota_sdt),
            "ident": ident,
            "wmat": wmat,
            "bias": bias,
        })
    return sched, in_maps


# --------------------------------------------------------------------------
# Device program
# --------------------------------------------------------------------------

def build_program(sched, cfg: Cfg):
    N, F, C, G = cfg.N, cfg.F, cfg.C, cfg.G
    NT, PADR, TR, QR, TC = cfg.NT, cfg.PAD, cfg.TR, cfg.QR, cfg.TC
    TS, GCOLS = sched["TS"], sched["GCOLS"]
    SDT = cfg.SDT
    f32 = mybir.dt.float32

    nc = bacc.Bacc(None, target_bir_lowering=False, num_devices=C,
                   dynamic_dma_scratch_size=cfg.dma_scratch,
                   num_swdge_queues=cfg.swdge_queues)

    # I/O
    x_in = nc.dram_tensor("x_arr", [P, NT * F], f32, kind="ExternalInput")
    gidx_in = nc.dram_tensor("gidx", [P, GCOLS], mybir.dt.int16, kind="ExternalInput")
    dstloc_in = nc.dram_tensor("dstloc", [P, TS], f32, kind="ExternalInput")
    dinvt_in = nc.dram_tensor("dinvt", [P, NT], f32, kind="ExternalInput")
    batchloc_in = nc.dram_tensor("batchloc", [P, NT], f32, kind="ExternalInput")
    invcnt_in = nc.dram_tensor("invcnt", [G, 1], f32, kind="ExternalInput")
    iota_in = nc.dram_tensor("iota", [P, P], f32, kind="ExternalInput")
    iota_sdt_in = nc.dram_tensor("iota_sdt", [P, P], SDT, kind="ExternalInput")
    ident_in = nc.dram_tensor("ident", [P, P], f32, kind="ExternalInput")
    wmat_in = nc.dram_tensor("wmat", [F, 3 * F], f32, kind="ExternalInput")
    bias_in = nc.dram_tensor("bias", [P, 3 * F], f32, kind="ExternalInput")
    out_dram = nc.dram_tensor("out", [G, cfg.OUT], f32, kind="ExternalOutput")

    # internal DRAM: one bounce + Shared table per quarter-shard
    QTILES = cfg.qtiles
    QBt = [0]
    for nt_j in QTILES:
        QBt.append(QBt[-1] + nt_j)
    bounce = [nc.dram_tensor(f"bounce{j}", [QTILES[j] * P, TC], SDT)
              if QTILES[j] else None for j in range(cfg.NQ)]
    # double-buffered per layer parity: superblock ordering ships next-layer
    # quarters while this layer still gathers from its own set
    tables = [[nc.dram_tensor(f"table{s}_{j}", [C * QTILES[j] * P, TC], SDT,
                              addr_space="Shared")
               if QTILES[j] else None for j in range(cfg.NQ)]
              for s in range(2)]
    pool_in = nc.dram_tensor("pool_in", [G, F], f32)
    pool_out = nc.dram_tensor("pool_out", [G, F], f32, addr_space="Shared")

    with tile.TileContext(nc) as tc:
        with (
            tc.tile_pool(name="state", bufs=1) as state,
            tc.tile_pool(name="gbuf", bufs=3) as gbuf,
            tc.tile_pool(name="spool", bufs=2) as spool,
            tc.tile_pool(name="sbt", bufs=2) as sbt,
            tc.tile_pool(name="tmp", bufs=4) as tmp,
            tc.tile_pool(name="ps_agg", bufs=4, space="PSUM") as ps_agg,
            tc.tile_pool(name="ps_t", bufs=2, space="PSUM") as ps_t,
            tc.tile_pool(name="ps_mm", bufs=2, space="PSUM") as ps_mm,
        ):
            # persistent state
            o_shard = state.tile([P, NT * F], f32, tag="o_shard")
            hw_stage = state.tile([P, NT * TC], SDT, tag="hw_stage")
            gidx_sb = state.tile([P, GCOLS], mybir.dt.int16, tag="gidx")
            dstloc_sb = state.tile([P, TS], f32, tag="dstloc")
            dinvt_sb = state.tile([P, NT], f32, tag="dinvt")
            batchloc_sb = state.tile([P, NT], f32, tag="batchloc")
            invcnt_sb = state.tile([G, 1], f32, tag="invcnt")
            iota_sb = state.tile([P, P], f32, tag="iota")
            iota_sdt_sb = state.tile([P, P], SDT, tag="iota_sdt")
            ident_sb = state.tile([P, P], f32, tag="ident")
            wmat_sb = state.tile([F, 3 * F], f32, tag="wmat")
            bias_sb = state.tile([P, 3 * F], f32, tag="bias")

            nc.gpsimd.load_library(library_config.mlp)
            if TC != F:
                nc.vector.memset(hw_stage[:], 0.0)
            nc.sync.dma_start(out=o_shard[:], in_=x_in[:])
            nc.sync.dma_start(out=gidx_sb[:], in_=gidx_in[:])
            nc.sync.dma_start(out=dstloc_sb[:], in_=dstloc_in[:])
            nc.sync.dma_start(out=dinvt_sb[:], in_=dinvt_in[:])
            nc.sync.dma_start(out=batchloc_sb[:], in_=batchloc_in[:])
            nc.sync.dma_start(out=invcnt_sb[:], in_=invcnt_in[:])
            nc.sync.dma_start(out=iota_sb[:], in_=iota_in[:])
            nc.sync.dma_start(out=iota_sdt_sb[:], in_=iota_sdt_in[:])
            nc.sync.dma_start(out=ident_sb[:], in_=ident_in[:])
            nc.sync.dma_start(out=wmat_sb[:], in_=wmat_in[:])
            nc.sync.dma_start(out=bias_sb[:], in_=bias_in[:])

            sub_q, sub_w = sched["sub_q"], sched["sub_w"]
            sub_first, sub_last = sched["sub_first"], sched["sub_last"]
            final_q = sched["final_q"]
            iota3 = iota_sdt_sb[:].rearrange("p (o f) -> p o f", o=1)

            def gemm_tile(layer, t):
                """hw_stage[t] = dinv * (o_shard[t] @ W_layer) as table rows."""
                o_t = o_shard[:, t * F:(t + 1) * F]
                psT = ps_t.tile([F, P], f32, tag="psT")
                nc.tensor.transpose(psT[:], o_t, ident_sb[:])
                sT = sbt.tile([F, P], f32, tag="sT")
                nc.vector.tensor_copy(sT[:], psT[:])
                psG = ps_mm.tile([P, F], f32, tag="psG")
                nc.tensor.matmul(
                    psG[:], lhsT=sT[:],
                    rhs=wmat_sb[:, layer * F:(layer + 1) * F],
                    start=True, stop=True)
                hw_t = hw_stage[:, t * TC:t * TC + F]
                nc.vector.tensor_scalar_mul(hw_t, psG[:], dinvt_sb[:, t:t + 1])

            def ship_quarter(j, tset):
                """DMA hw_stage quarter j to DRAM and AllGather into table j."""
                nt_j = QTILES[j]
                if not nt_j:
                    return
                hw_q = hw_stage[:, QBt[j] * TC:QBt[j + 1] * TC]
                nc.sync.dma_start(
                    out=bounce[j].ap().rearrange("(t p) c -> p t c", p=P),
                    in_=hw_q.rearrange("p (t c) -> p t c", c=TC))
                nc.gpsimd.collective_compute(
                    "AllGather", mybir.AluOpType.bypass,
                    replica_groups=[list(range(C))],
                    ins=[bounce[j].ap().opt()],
                    outs=[tables[tset][j].ap().opt()])

            def quarter_of_tile(t):
                for j in range(cfg.NQ):
                    if QBt[j] <= t < QBt[j + 1]:
                        return j
                raise AssertionError(t)

            # conv-0 tables from x
            for j in range(cfg.NQ):
                for t in range(QBt[j], QBt[j + 1]):
                    gemm_tile(0, t)
                ship_quarter(j, 0)

            pool_state = dict(psP=None, closed=0)

            def finalize_tile(layer, w):
                """All 4 passes of `layer` accumulated into o_shard[w]:
                epilogue, then feed forward (next GEMM+ship, or pooling)."""
                o_t = o_shard[:, w * F:(w + 1) * F]
                tt = tmp.tile([P, F], f32, tag="ep")
                nc.vector.tensor_scalar_mul(tt[:], o_t, dinvt_sb[:, w:w + 1])
                if layer == 0:
                    nc.vector.tensor_tensor(
                        tt[:], tt[:], bias_sb[:, layer * F:(layer + 1) * F],
                        op=mybir.AluOpType.add)
                    nc.vector.tensor_scalar_max(o_t, tt[:], 0.0)
                else:
                    nc.vector.tensor_tensor(
                        o_t, tt[:], bias_sb[:, layer * F:(layer + 1) * F],
                        op=mybir.AluOpType.add)
                if layer < 2:
                    gemm_tile(layer + 1, w)
                    jq = quarter_of_tile(w)
                    quarter_fill[jq] += 1
                    if quarter_fill[jq] == QTILES[jq]:
                        # defer the AllGather a few gather calls so the Pool
                        # sequencer doesn't stall desc-gen waiting on the
                        # GEMM/DMA pipeline to drain
                        pending_ships.append(
                            [cfg.ship_delay, jq, (layer + 1) % 2])
                else:
                    Gt = spool.tile([P, G], f32, tag="Gt")
                    nc.vector.tensor_scalar(
                        Gt[:], iota_sb[:, :G], batchloc_sb[:, w:w + 1], None,
                        op0=mybir.AluOpType.is_equal)
                    if pool_state["psP"] is None:
                        pool_state["psP"] = ps_mm.tile(
                            [G, F], f32, tag="psG", name="psP")
                    pool_state["closed"] += 1
                    nc.tensor.matmul(
                        pool_state["psP"][:], lhsT=Gt[:], rhs=o_t,
                        start=(pool_state["closed"] == 1),
                        stop=(pool_state["closed"] == NT))

            pending_ships = []

            def tick_ships(force=False):
                for ent in pending_ships:
                    ent[0] -= 1
                while pending_ships and (force or pending_ships[0][0] <= 0):
                    _, jq, tset = pending_ships.pop(0)
                    ship_quarter(jq, tset)

            for layer in range(3):
                win_psum = None
                win_init = np.zeros(NT, dtype=bool)
                quarter_fill = [0] * cfg.NQ
                for ci, call in enumerate(sched["calls"]):
                    tick_ships()
                    n, gs0, qq = call["n"], call["gs0"], call["q"]
                    gt = gbuf.tile([P, cfg.GCH * TC], SDT, tag="gt")
                    idxs_ap = gidx_sb[:, 8 * gs0:8 * (gs0 + n)]
                    nc.gpsimd.dma_gather(
                        gt[:].rearrange("p (n c) -> p n c", c=TC)[:, :n, :],
                        tables[layer % 2][qq][:, :],
                        idxs_ap,
                        n * P, n * P, TC,
                        single_packet=cfg.single_packet,
                        queue_num=ci % cfg.swdge_queues)
                    # one-hot selection matrices for the whole call, one DVE op
                    S_b = spool.tile([P, cfg.GCH * P], SDT, tag="S")
                    nc.vector.tensor_tensor(
                        S_b[:, :n * P].rearrange("p (n f) -> p n f", f=P),
                        dstloc_sb[:, gs0:gs0 + n].to_broadcast([P, n, P]),
                        iota3.to_broadcast([P, n, P]),
                        op=mybir.AluOpType.is_equal)
                    for j in range(n):
                        gs = gs0 + j
                        w = int(sub_w[gs])
                        if sub_first[gs]:
                            win_psum = ps_agg.tile([P, F], f32, tag="agg")
                        nc.tensor.matmul(
                            win_psum[:], lhsT=S_b[:, j * P:(j + 1) * P],
                            rhs=gt[:, j * TC:j * TC + F],
                            start=bool(sub_first[gs]), stop=bool(sub_last[gs]))
                        if sub_last[gs]:
                            o_w = o_shard[:, w * F:(w + 1) * F]
                            if not win_init[w]:
                                nc.vector.tensor_copy(o_w, win_psum[:])
                                win_init[w] = True
                            else:
                                nc.vector.tensor_tensor(
                                    o_w, o_w, win_psum[:],
                                    op=mybir.AluOpType.add)
                            if qq == final_q[w]:
                                finalize_tile(layer, w)

                tick_ships(force=True)

            # ---- pooled sums across cores
            sums = tmp.tile([G, F], f32, tag="sums")
            nc.vector.tensor_copy(sums[:], pool_state["psP"][:])
            nc.sync.dma_start(out=pool_in[:, :], in_=sums[:])
            nc.gpsimd.collective_compute(
                "AllReduce", mybir.AluOpType.add,
                replica_groups=[list(range(C))],
                ins=[pool_in.ap().opt()],
                outs=[pool_out.ap().opt()])
            sums2 = tmp.tile([G, F], f32, tag="sums")
            nc.sync.dma_start(out=sums2[:], in_=pool_out[:, :])
            res = tmp.tile([G, cfg.OUT], f32, tag="res")
            nc.vector.tensor_scalar_mul(res[:], sums2[:, :cfg.OUT], invcnt_sb[:])
            nc.sync.dma_start(out=out_dram[:, :], in_=res[:])

    return nc


# --------------------------------------------------------------------------
# Entry point
# --------------------------------------------------------------------------

def _install_trace_hooks():
    """The agent image's antenv lacks axon_hooks; reconstruct it so
    run_bass_kernel_spmd(trace=True) can NTFF-profile via ctypes, and stub
    the S3 artifact upload."""
    import types
    import antenv
    if "antenv.axon_hooks" not in sys.modules:
        mod = types.ModuleType("antenv.axon_hooks")
        mod._hook = None
        def _set(h):
            mod._hook = h
        def _get():
            return mod._hook
        mod.set_axon_ntff_profile_hook = _set
        mod.get_axon_ntff_profile_hook = _get
        sys.modules["antenv.axon_hooks"] = mod
        antenv.axon_hooks = mod
    hooks = sys.modules["antenv.axon_hooks"]
    if hooks.get_axon_ntff_profile_hook() is None:
        if "/root/.axon_site" not in sys.path:
            sys.path.insert(0, "/root/.axon_site")
        from trn_agent_boot.trn_boot import _ntff_profile_via_ctypes
        hooks.set_axon_ntff_profile_hook(
            _ntff_profile_via_ctypes("/opt/axon/libaxon_pjrt.so"))
    import concourse.bass_utils as bu
    bu.upload_artifacts = lambda tmpdir: tmpdir


def kernel(x, edge_index, batch, num_graphs, W1, b1, W2, b2, W3, b3,
           _trace=False, _cfg=None):
    cfg = _cfg or FULL
    assert int(num_graphs) == cfg.G
    sched, in_maps = host_prep(x, edge_index, batch, W1, b1, W2, b2, W3, b3, cfg)
    nc = build_program(sched, cfg)
    nc.finalize()

    if _trace:
        _install_trace_hooks()
    from concourse.bass_utils import run_bass_kernel_spmd
    res = run_bass_kernel_spmd(nc, in_maps, core_ids=list(range(cfg.C)),
                               trace=_trace)
    out = np.asarray(res.results[0]["out"], dtype=np.float32)
    if _trace:
        return out, res.exec_time_ns
    return out



# revision 43
# speedup vs baseline: 2.3245x; 2.3245x over previous
"""Trainium2 Bass kernel for a 3-layer GCN (nn_GCN_37383395344580).

Strategy (8 NeuronCores, one SPMD program):
  - Nodes are dealt by in-degree across 8 cores x 98 windows of 128 dst
    slots; a per-(window,lane) greedy pass then permutes the 8 co-ranked
    nodes across cores to balance per-(core,window,src-quarter) edge counts
    (minimizing SPMD max-over-cores padding AND ceil-to-128 rounding).
  - norm factorizes: norm(s,d) = dinv[s]*dinv[d], so messages are rows of a
    replicated bf16 "table" T = dinv * (h @ W) and aggregated sums are
    rescaled by dinv[d]: zero per-edge vector work.
  - Self loops never enter the gather stream: their contribution
    dinv[d]^2 * T-ish is added per-window from the locally staged table
    (hw_stage) with a per-partition scalar multiply on the Scalar engine.
  - Per layer: per-window GEMM + row scale feed 4 quarter-shard AllGathers
    (pipelined with the previous layer's gather passes); gather passes of
    dma_gather (int16 indices address one quarter table, 256B rows) spread
    round-robin over all 4 SWDGE queues (each queue has its own Q7 CPU pair
    doing descriptor generation -- 4 concurrent desc-gen streams); one
    batched is_equal builds up to 32 one-hot selection matrices per DVE op
    (2x DVE mode via a pair-duplicated bf16 dstloc layout); window matmuls
    (edges = contraction dim) accumulate [128 dst x 64] slices of a
    persistent per-superblock PSUM accumulator -- each window's 4 quarter
    passes accumulate in PSUM with no intermediate SBUF round-trips.
  - 8 superblocks of 12-13 windows; quarter j's next-layer table ships
    after superblock 2j+1 closes, keeping AllGathers ~2 superblocks ahead
    of the gather passes that consume them.
  - Final: one-hot graph-id matmuls pool per-graph sums, AllReduce across
    cores, scale by host-computed 1/max(cnt,1).

Hardware notes learned on TRN2:
  - dma_gather/dma_scatter_add need gpsimd.load_library(library_config.mlp).
  - single_packet=True hangs beyond ~1024 indices/call; use
    single_packet=False for large calls.
  - Q7 SWDGE desc-gen (~5-7ns/row per queue pair) is the main cost; queue q
    runs on Q7 cpu pair (2q, 2q+1), so 4 queues give 4x desc-gen.
  - PSUM pool tiles are rounded up to whole 2KB banks; keep accumulators at
    exact bank multiples.
"""

import os
import sys
from dataclasses import dataclass

import numpy as np

for _p in ("/opt/trn_rl_repo",):
    if _p not in sys.path and os.path.isdir(_p):
        sys.path.insert(0, _p)

import concourse.bass as bass
import concourse.bacc as bacc
import concourse.tile as tile
from concourse import library_config, mybir

P = 128  # partitions


@dataclass(frozen=True)
class Cfg:
    N: int = 100000       # nodes
    F: int = 64           # feature width (all layers; layer-3 W padded)
    OUT: int = 32         # final feature width
    G: int = 64           # graphs
    C: int = 8            # cores
    NQ: int = 4           # gather quadrants (int16 index limit)
    NSB: int = 8          # superblocks (PSUM accumulator groups)
    GCH: int = 16         # max subchunks (of 128 edges) per dma_gather call
    table_bf16: bool = True  # bf16 gather table (half AllGather bytes)
    dma_scratch: int = 16384  # SWDGE descriptor carveout bytes/partition
    single_packet: bool = False
    swdge_queues: int = 4
    ship_delay: int = 6   # gather calls between quarter-GEMM done and its AG

    @property
    def NLOC(self):
        assert self.N % self.C == 0
        return self.N // self.C

    @property
    def NT(self):
        return -(-self.NLOC // P)

    @property
    def PAD(self):
        return self.NT * P

    @property
    def TR(self):
        return self.C * self.PAD

    @property
    def QR(self):
        assert self.TR % self.NQ == 0
        return self.TR // self.NQ

    @property
    def TC(self):  # table row width in elements (row stride must be 256B)
        return 2 * self.F if self.table_bf16 else self.F

    @property
    def QROWS(self):
        """Local table rows per quarter (row-granular, NOT window-aligned:
        equal quarters keep per-(window,src-quarter) edge counts feasible
        for 4 subchunks). C*QROWS must fit int16 gather indices."""
        assert self.PAD % self.NQ == 0
        qr = self.PAD // self.NQ
        assert self.C * qr <= 32768
        return qr

    @property
    def sbtiles(self):
        """Window tiles per superblock (NSB groups of consecutive windows)."""
        base = [self.NT // self.NSB] * self.NSB
        for i in range(self.NT % self.NSB):
            base[i] += 1
        return base

    @property
    def SBW(self):
        """Max windows per superblock; PSUM acc 64*SBW f32 must be bank mult."""
        m = max(self.sbtiles)
        # round acc width up to a whole number of 2KB banks (512 f32)
        return -(-m * self.F // 512) * 512 // self.F

    @property
    def SDT(self):
        return mybir.dt.bfloat16 if self.table_bf16 else mybir.dt.float32


FULL = Cfg()


# --------------------------------------------------------------------------
# Host-side schedule + per-core stream construction (pure numpy)
# --------------------------------------------------------------------------

def node_placement(src, dst, cfg: Cfg):
    """Two-pass placement.

    Pass 1: degree-rank round-robin fixes each node's QUARTER (the QROWS-row
    table index range it lives in; quarters get EQUAL node counts N/NQ),
    which determines the src-quarter of every edge.  Pass 2 then, per
    quarter, greedily assigns nodes to (core, window-segment, lane) slots
    balancing per-(core,window) in-degree-by-src-quarter counts, followed by
    targeted swap polish that pushes cells under the 4-subchunk (512-edge)
    boundary wherever feasible.  Quarters are row-granular: a boundary may
    cut a window in half (lanes split between quarters).
    """
    N, C, NT, NQ = cfg.N, cfg.C, cfg.NT, cfg.NQ
    QR = cfg.QROWS

    s = np.asarray(src, dtype=np.int64)
    d = np.asarray(dst, dtype=np.int64)

    deg = np.bincount(d, minlength=N)
    order = np.argsort(-deg, kind="stable")
    rank = np.empty(N, dtype=np.int64)
    rank[order] = np.arange(N)
    q_of_node = (rank % NQ).astype(np.int64)

    # quarter-pair balance: swap nodes across quarters until every cell of
    # M[src_quarter, dst_quarter] is under cap (otherwise some dst quarter
    # receives > cap edges from one src quarter, forcing a 5th subchunk on
    # all of its windows)
    CAP = -(-len(s) // (NQ * NQ))
    for _ in range(3):
        O = np.zeros((N, NQ), np.int64)
        I = np.zeros((N, NQ), np.int64)
        np.add.at(O, (s, q_of_node[d]), 1)
        np.add.at(I, (d, q_of_node[s]), 1)
        M = np.zeros((NQ, NQ), np.int64)
        np.add.at(M, (q_of_node[s], q_of_node[d]), 1)
        improved = 0
        for _it in range(400):
            over = M - CAP
            if over.max() <= 0:
                break
            qa, qb = np.unravel_index(np.argmax(over), over.shape)
            best = None
            uq = np.nonzero(q_of_node == qa)[0]
            us = uq[np.argsort(-O[uq, qb])[:16]]
            for qc in range(NQ):
                if qc == qa:
                    continue
                wq = np.nonzero(q_of_node == qc)[0]
                ws = wq[np.argsort(O[wq, qb])[:16]]
                for u in us:
                    for w in ws:
                        dO = O[w] - O[u]
                        dI = I[w] - I[u]
                        M2 = M.copy()
                        M2[qa] += dO
                        M2[qc] -= dO
                        M2[:, qa] += dI
                        M2[:, qc] -= dI
                        phi = (np.maximum(M2 - CAP, 0) ** 2).sum()
                        if best is None or phi < best[0]:
                            best = (phi, u, w, qc, M2)
            phi0 = (np.maximum(M - CAP, 0) ** 2).sum()
            if best is None or best[0] >= phi0:
                break
            _, u, w, qc, M = best
            q_of_node[u], q_of_node[w] = qc, qa
            improved += 1
        if improved == 0:
            break

    # per-dst-node in-degree by src quarter: L[v, q']
    Lq = np.zeros((N, NQ), dtype=np.int64)
    np.add.at(Lq, (d, q_of_node[s]), 1)

    def polish(groups, loads, limits, rounds=10):
        """Targeted swaps: for any (group a, component qp) with
        loads[a,qp] > limits[a], swap a node of a with a node of another
        group so the overflow shrinks and no group exceeds its limit."""
        G = len(groups)
        for _ in range(rounds):
            moved = False
            for a in range(G):
                for qp in range(NQ):
                    guard = 0
                    while loads[a, qp] > limits[a] and guard < 8:
                        guard += 1
                        need = loads[a, qp] - limits[a]
                        A = np.asarray(groups[a])
                        ai = np.arange(len(A))
                        # candidates: top-32 donors by L[qp]
                        if len(A) > 32:
                            ai = np.argsort(-Lq[A, qp], kind="stable")[:32]
                            A = A[ai]
                        La = Lq[A]
                        # candidate partner groups: 3 least loaded on qp
                        others = [b for b in range(G) if b != a and groups[b]]
                        others.sort(key=lambda b: loads[b, qp])
                        done = False
                        for b in others[:6]:
                            B = np.asarray(groups[b])
                            bi = np.arange(len(B))
                            # candidates: bottom-32 receivers by L[qp]
                            if len(B) > 32:
                                bi = np.argsort(Lq[B, qp], kind="stable")[:32]
                                B = B[bi]
                            Lb = Lq[B]
                            dlt = La[:, None, :] - Lb[None, :, :]  # [na,nb,4]
                            prog = dlt[:, :, qp]
                            feas = (prog > 0)
                            feas &= np.all(
                                loads[b][None, None, :] + dlt
                                <= limits[b], axis=2)
                            feas &= np.all(
                                loads[a][None, None, :] - dlt
                                <= limits[a], axis=2)
                            if not feas.any():
                                continue
                            score = np.where(
                                feas,
                                np.minimum(prog, need)
                                - 0.01 * np.maximum(prog - need, 0),
                                -np.inf)
                            ui, vi = np.unravel_index(
                                np.argmax(score), score.shape)
                            u, v = int(A[ui]), int(B[vi])
                            groups[a][int(ai[ui])] = v
                            groups[b][int(bi[vi])] = u
                            loads[a] += Lq[v] - Lq[u]
                            loads[b] += Lq[u] - Lq[v]
                            moved = True
                            done = True
                            break
                        if not done:
                            break
            if not moved:
                break

    node_core = np.empty(N, dtype=np.int64)
    node_l = np.empty(N, dtype=np.int64)
    for q in range(NQ):
        nodes_q = np.nonzero(q_of_node == q)[0]
        r0, r1 = q * QR, (q + 1) * QR
        segs = []  # (window, lane_lo, lane_hi)
        for w in range(r0 // P, (r1 - 1) // P + 1):
            lo = max(r0, w * P) - w * P
            hi = min(r1, (w + 1) * P) - w * P
            segs.append((w, lo, hi))
        nseg = len(segs)
        lanes_seg = np.array([hi - lo for (_, lo, hi) in segs])
        segcap = lanes_seg * C
        tot = Lq[nodes_q].sum(axis=1)
        order_q = nodes_q[np.argsort(-tot, kind="stable")]
        # stage 1: nodes -> window segments, equalizing scaled totals
        wt = np.zeros((nseg, NQ), dtype=np.float64)
        wfill = np.zeros(nseg, dtype=np.int64)
        scale = (P * C) / segcap.astype(np.float64)
        win_of = np.empty(len(order_q), dtype=np.int64)
        for i, v in enumerate(order_q):
            lv = Lq[v]
            nt_ = wt + lv[None, :] * scale[:, None]
            key = (nt_ * nt_).sum(axis=1)
            key[wfill >= segcap] = np.inf
            b = int(np.argmin(key))
            wt[b] += lv * scale[b]
            win_of[i] = b
            wfill[b] += 1
        # stage 1 polish: totals under 4*C*lanes - 6 (slack for stage 2)
        seg_nodes = [list(order_q[win_of == b]) for b in range(nseg)]
        totU = np.zeros((nseg, NQ), dtype=np.int64)
        for b in range(nseg):
            totU[b] = Lq[seg_nodes[b]].sum(axis=0)
        polish(seg_nodes, totU, 4 * C * lanes_seg - 6)
        # stage 2: per segment, nodes -> cores
        for b in range(nseg):
            w, lo, hi = segs[b]
            lanes = hi - lo
            lim = 4 * lanes
            sel = seg_nodes[b]
            cc = np.zeros((C, NQ), dtype=np.int64)
            cfill = np.zeros(C, dtype=np.int64)
            core_of = {}
            for v in sel:
                lv = Lq[v]
                nc_ = cc + lv[None, :]
                key = (nc_ * nc_).sum(axis=1).astype(np.float64) \
                    + np.maximum(nc_ - lim, 0).sum(axis=1) * 1e9
                key[cfill >= lanes] = np.inf
                ci = int(np.argmin(key))
                cc[ci] += lv
                core_of[v] = ci
                cfill[ci] += 1
            by_core = [[v for v in sel if core_of[v] == ci] for ci in range(C)]
            polish(by_core, cc, np.full(C, lim, dtype=np.int64), rounds=8)
            for ci in range(C):
                for k, v in enumerate(by_core[ci]):
                    node_core[v] = ci
                    node_l[v] = w * P + lo + k
    return node_core, node_l


def build_schedule(src, dst, cfg: Cfg):
    """src/dst WITHOUT self loops. Returns (sched, percore_gidx,
    percore_dstloc, node_core, node_l)."""
    N, C, NQ, NSB = cfg.N, cfg.C, cfg.NQ, cfg.NSB
    NT = cfg.NT
    QR = cfg.QROWS
    SBTILES = cfg.sbtiles

    s = np.asarray(src, dtype=np.int64)
    d = np.asarray(dst, dtype=np.int64)
    node_core, node_l = node_placement(s, d, cfg)

    l_s = node_l[s]
    q = l_s // QR
    gidx_val = (node_core[s] * QR + (l_s - q * QR)).astype(np.int16)

    c = node_core[d]
    dl = node_l[d]
    w = dl // P
    dloc = dl % P

    # window -> superblock, and position within superblock
    SBB = np.concatenate([[0], np.cumsum(SBTILES)])  # window bounds per sb
    win_sb = np.searchsorted(SBB, np.arange(NT), side="right") - 1
    win_pos = np.arange(NT) - SBB[win_sb]

    # Run order: for each superblock, 4 src-quarter passes over its windows.
    # (Pass-major across superblock pairs was tried and measured slower: the
    # pair-boundary finalize burst outweighs the longer AllGather deadline.)
    NR = NQ * NT
    run_q = np.empty(NR, dtype=np.int64)
    run_w = np.empty(NR, dtype=np.int64)
    runpos = np.empty((NQ, NT), dtype=np.int64)
    r = 0
    for sb in range(NSB):
        ws = np.arange(SBB[sb], SBB[sb + 1])
        for qq in range(NQ):
            for w_ in ws:
                run_q[r] = qq
                run_w[r] = w_
                runpos[qq, w_] = r
                r += 1
    assert r == NR

    key = c * NR + runpos[q, w]
    counts = np.bincount(key, minlength=C * NR).reshape(C, NR)
    nsub = -(-counts.max(axis=0) // P)          # [NR] in run order
    sub_base = np.zeros(NR + 1, dtype=np.int64)
    np.cumsum(nsub, out=sub_base[1:])
    TS = int(sub_base[-1])
    SLOTS = TS * P

    r_of_sub = np.searchsorted(sub_base, np.arange(TS), side="right") - 1
    sub_q = run_q[r_of_sub]
    sub_w = run_w[r_of_sub]
    # accumulation start/stop: window's first/last nonempty run in run order
    first_run = np.full(NT, -1, dtype=np.int64)
    last_run = np.full(NT, -1, dtype=np.int64)
    for w_i in range(NT):
        rs = runpos[:, w_i]
        nz = rs[nsub[rs] > 0]
        if len(nz):
            nz_sorted = np.sort(nz)
            first_run[w_i] = nz_sorted[0]
            last_run[w_i] = nz_sorted[-1]
    sub_first = np.zeros(TS, dtype=bool)
    sub_last = np.zeros(TS, dtype=bool)
    for w_i in range(NT):
        if first_run[w_i] >= 0:
            sub_first[sub_base[first_run[w_i]]] = True
            sub_last[sub_base[last_run[w_i] + 1] - 1] = True

    # gather calls: contiguous same-q subchunk segments, split at GCH
    calls = []
    seg = []
    for rr in range(NR):
        if seg and run_q[rr] != run_q[seg[-1]]:
            lo, hi = int(sub_base[seg[0]]), int(sub_base[seg[-1] + 1])
            gs0 = lo
            while gs0 < hi:
                n = min(cfg.GCH, hi - gs0)
                calls.append(dict(q=int(run_q[seg[0]]), gs0=gs0, n=n))
                gs0 += n
            seg = []
        seg.append(rr)
    if seg:
        lo, hi = int(sub_base[seg[0]]), int(sub_base[seg[-1] + 1])
        gs0 = lo
        while gs0 < hi:
            n = min(cfg.GCH, hi - gs0)
            calls.append(dict(q=int(run_q[seg[0]]), gs0=gs0, n=n))
            gs0 += n
    calls = [cl for cl in calls if cl["n"] > 0]
    GCOLS = SLOTS // 16

    order = np.argsort(key, kind="stable")
    key_sorted = key[order]
    run_first_idx = np.searchsorted(key_sorted, np.arange(C * NR), side="left")
    edge_order_pos = np.empty(len(s), dtype=np.int64)
    edge_order_pos[order] = np.arange(len(s)) - run_first_idx[key_sorted]

    slot = sub_base[key % NR] * P + edge_order_pos

    percore_gidx = []
    percore_dstloc = []
    for cc in range(C):
        m = c == cc
        gfull = np.zeros(SLOTS, dtype=np.int16)
        dfull = np.full(SLOTS, -1.0, dtype=np.float64)
        gfull[slot[m]] = gidx_val[m]
        dfull[slot[m]] = dloc[m]
        packed = np.ascontiguousarray(
            np.tile(gfull.reshape(GCOLS, 16).T, (8, 1)))
        percore_gidx.append(packed)
        # dstloc: [P, TS, 2] duplicated pairs in bf16 (enables DVE 2x mode)
        dl_mat = dfull.reshape(TS, P).T  # [P, TS]
        dl_pairs = np.repeat(dl_mat[:, :, None], 2, axis=2).reshape(P, 2 * TS)
        percore_dstloc.append(np.ascontiguousarray(dl_pairs))

    sched = dict(TS=TS, GCOLS=GCOLS, calls=calls, sub_q=sub_q, sub_w=sub_w,
                 sub_first=sub_first, sub_last=sub_last,
                 first_run=first_run, last_run=last_run,
                 win_sb=win_sb, win_pos=win_pos, SBB=SBB)
    return sched, percore_gidx, percore_dstloc, node_core, node_l


def host_prep(x, edge_index, batch, W1, b1, W2, b2, W3, b3, cfg: Cfg):
    """Build in_maps (list of dicts per core)."""
    N, F, C, G = cfg.N, cfg.F, cfg.C, cfg.G
    NT = cfg.NT
    PADR = cfg.PAD

    e0 = np.asarray(edge_index[0], dtype=np.int64)
    e1 = np.asarray(edge_index[1], dtype=np.int64)

    # degree INCLUDES the self loop (A+I normalization)
    deg = (np.bincount(e1, minlength=N) + 1).astype(np.float64)
    dinv = (1.0 / np.sqrt(deg)).astype(np.float32)

    sched, percore_gidx, percore_dstloc, node_core, node_l = \
        build_schedule(e0, e1, cfg)

    batch = np.asarray(batch, dtype=np.int64)
    cnts = np.bincount(batch, minlength=G).astype(np.float64)
    invcnt = (1.0 / np.maximum(cnts, 1.0)).astype(np.float32)[:, None]

    W3p = np.zeros((F, F), np.float32)
    W3p[:, :cfg.OUT] = np.asarray(W3, np.float32)
    b3p = np.zeros((F,), np.float32)
    b3p[:cfg.OUT] = np.asarray(b3, np.float32)
    wmat = np.concatenate([np.asarray(W1, np.float32),
                           np.asarray(W2, np.float32), W3p], axis=1)
    bias = np.broadcast_to(
        np.concatenate([np.asarray(b1, np.float32),
                        np.asarray(b2, np.float32), b3p])[None, :], (P, 3 * F)
    ).copy()

    iota_f32 = np.broadcast_to(np.arange(P, dtype=np.float32)[None, :], (P, P)).copy()
    ident = np.eye(P, dtype=np.float32)

    import ml_dtypes
    iota_sdt = iota_f32.astype(ml_dtypes.bfloat16) if cfg.table_bf16 else iota_f32

    x = np.asarray(x, np.float32)
    in_maps = []
    for cc in range(C):
        m = node_core == cc
        ls = node_l[m]
        xs = np.zeros((PADR, F), np.float32)
        xs[ls] = x[m]
        x_arr = np.ascontiguousarray(
            xs.reshape(NT, P, F).transpose(1, 0, 2).reshape(P, NT * F))

        dv = np.zeros((PADR,), np.float32)
        dv[ls] = dinv[m]
        dinvt = np.ascontiguousarray(dv.reshape(NT, P).T)

        bl = np.full((PADR,), -1.0, np.float32)
        bl[ls] = batch[m].astype(np.float32)
        batchloc = np.ascontiguousarray(bl.reshape(NT, P).T)

        dstloc2 = percore_dstloc[cc].astype(ml_dtypes.bfloat16) \
            if cfg.table_bf16 else percore_dstloc[cc].astype(np.float32)

        in_maps.append({
            "x_arr": x_arr,
            "gidx": percore_gidx[cc],
            "dstloc2": np.ascontiguousarray(dstloc2),
            "dinvt": dinvt,
            "batchloc": batchloc,
            "invcnt": invcnt,
            "iota": iota_f32,
            "iota_sdt": np.ascontiguousarray(iota_sdt),
            "ident": ident,
            "wmat": wmat,
            "bias": bias,
        })
    return sched, in_maps


# --------------------------------------------------------------------------
# Device program
# --------------------------------------------------------------------------

def build_program(sched, cfg: Cfg):
    N, F, C, G = cfg.N, cfg.F, cfg.C, cfg.G
    NT, TC = cfg.NT, cfg.TC
    NSB, SBW = cfg.NSB, cfg.SBW
    TS, GCOLS = sched["TS"], sched["GCOLS"]
    SDT = cfg.SDT
    f32 = mybir.dt.float32

    nc = bacc.Bacc(None, target_bir_lowering=False, num_devices=C,
                   dynamic_dma_scratch_size=cfg.dma_scratch,
                   num_swdge_queues=cfg.swdge_queues)

    # I/O
    x_in = nc.dram_tensor("x_arr", [P, NT * F], f32, kind="ExternalInput")
    gidx_in = nc.dram_tensor("gidx", [P, GCOLS], mybir.dt.int16, kind="ExternalInput")
    dstloc_in = nc.dram_tensor("dstloc2", [P, 2 * TS], SDT, kind="ExternalInput")
    dinvt_in = nc.dram_tensor("dinvt", [P, NT], f32, kind="ExternalInput")
    batchloc_in = nc.dram_tensor("batchloc", [P, NT], f32, kind="ExternalInput")
    invcnt_in = nc.dram_tensor("invcnt", [G, 1], f32, kind="ExternalInput")
    iota_in = nc.dram_tensor("iota", [P, P], f32, kind="ExternalInput")
    iota_sdt_in = nc.dram_tensor("iota_sdt", [P, P], SDT, kind="ExternalInput")
    ident_in = nc.dram_tensor("ident", [P, P], f32, kind="ExternalInput")
    wmat_in = nc.dram_tensor("wmat", [F, 3 * F], f32, kind="ExternalInput")
    bias_in = nc.dram_tensor("bias", [P, 3 * F], f32, kind="ExternalInput")
    out_dram = nc.dram_tensor("out", [G, cfg.OUT], f32, kind="ExternalOutput")

    # internal DRAM: one bounce + Shared table per quarter-shard
    QR = cfg.QROWS
    bounce = [nc.dram_tensor(f"bounce{j}", [QR, TC], SDT)
              for j in range(cfg.NQ)]
    tables = [[nc.dram_tensor(f"table{s}_{j}", [C * QR, TC], SDT,
                              addr_space="Shared")
               for j in range(cfg.NQ)]
              for s in range(2)]
    # windows intersecting each quarter's row range (ship trigger sets)
    quarters_of_window = [[] for _ in range(NT)]
    qwin_count = [0] * cfg.NQ
    for j in range(cfg.NQ):
        for w in range(j * QR // P, ((j + 1) * QR - 1) // P + 1):
            quarters_of_window[w].append(j)
            qwin_count[j] += 1
    pool_in = nc.dram_tensor("pool_in", [G, F], f32)
    pool_out = nc.dram_tensor("pool_out", [G, F], f32, addr_space="Shared")

    SBB = sched["SBB"]
    win_sb = sched["win_sb"]
    win_pos = sched["win_pos"]

    with tile.TileContext(nc) as tc:
        with (
            tc.tile_pool(name="state", bufs=1) as state,
            tc.tile_pool(name="gbuf", bufs=12) as gbuf,
            tc.tile_pool(name="spool", bufs=6) as spool,
            tc.tile_pool(name="sbt", bufs=2) as sbt,
            tc.tile_pool(name="tmp", bufs=4) as tmp,
            tc.tile_pool(name="ps_acc", bufs=2, space="PSUM") as ps_acc,
            tc.tile_pool(name="ps_t", bufs=2, space="PSUM") as ps_t,
            tc.tile_pool(name="ps_mm", bufs=2, space="PSUM") as ps_mm,
        ):
            # persistent state
            o_shard = state.tile([P, NT * F], f32, tag="o_shard")
            hw_stage = state.tile([P, NT * TC], SDT, tag="hw_stage")
            gidx_sb = state.tile([P, GCOLS], mybir.dt.int16, tag="gidx")
            dstloc_sb = state.tile([P, 2 * TS], SDT, tag="dstloc")
            dinvt_sb = state.tile([P, NT], f32, tag="dinvt")
            batchloc_sb = state.tile([P, NT], f32, tag="batchloc")
            invcnt_sb = state.tile([G, 1], f32, tag="invcnt")
            iota_sb = state.tile([P, P], f32, tag="iota")
            iota_sdt_sb = state.tile([P, P], SDT, tag="iota_sdt")
            ident_sb = state.tile([P, P], f32, tag="ident")
            wmat_sb = state.tile([F, 3 * F], f32, tag="wmat")
            bias_sb = state.tile([P, 3 * F], f32, tag="bias")

            nc.gpsimd.load_library(library_config.mlp)
            if TC != F:
                nc.vector.memset(hw_stage[:], 0.0)
            nc.sync.dma_start(out=o_shard[:], in_=x_in[:])
            nc.sync.dma_start(out=gidx_sb[:], in_=gidx_in[:])
            nc.sync.dma_start(out=dstloc_sb[:], in_=dstloc_in[:])
            nc.sync.dma_start(out=dinvt_sb[:], in_=dinvt_in[:])
            nc.sync.dma_start(out=batchloc_sb[:], in_=batchloc_in[:])
            nc.sync.dma_start(out=invcnt_sb[:], in_=invcnt_in[:])
            nc.sync.dma_start(out=iota_sb[:], in_=iota_in[:])
            nc.sync.dma_start(out=iota_sdt_sb[:], in_=iota_sdt_in[:])
            nc.sync.dma_start(out=ident_sb[:], in_=ident_in[:])
            nc.sync.dma_start(out=wmat_sb[:], in_=wmat_in[:])
            nc.sync.dma_start(out=bias_sb[:], in_=bias_in[:])

            sub_q, sub_w = sched["sub_q"], sched["sub_w"]
            sub_first, sub_last = sched["sub_first"], sched["sub_last"]

            def gemm_tile(layer, t):
                """hw_stage[t] = dinv * (o_shard[t] @ W_layer) as table rows."""
                o_t = o_shard[:, t * F:(t + 1) * F]
                psT = ps_t.tile([F, P], f32, tag="psT")
                nc.tensor.transpose(psT[:], o_t, ident_sb[:])
                sT = sbt.tile([F, P], f32, tag="sT")
                nc.scalar.copy(sT[:], psT[:])
                psG = ps_mm.tile([P, F], f32, tag="psG")
                nc.tensor.matmul(
                    psG[:], lhsT=sT[:],
                    rhs=wmat_sb[:, layer * F:(layer + 1) * F],
                    start=True, stop=True)
                hw_t = hw_stage[:, t * TC:t * TC + F]
                nc.scalar.mul(hw_t, psG[:], dinvt_sb[:, t:t + 1])

            def bounce_quarter(j, tset):
                """DMA hw_stage rows [j*QR,(j+1)*QR) to the bounce buffer.
                Issued eagerly when the quarter's GEMMs complete, so the
                AllGather (deferred a few calls) never stalls the gpsimd
                queue waiting for it.  Quarter bounds may cut windows."""
                r0, r1 = j * QR, (j + 1) * QR
                t0 = r0 // P
                if r0 % P:                      # head: partial window t0
                    lo = r0 % P
                    nc.sync.dma_start(
                        out=bounce[j].ap()[0:P - lo, :],
                        in_=hw_stage[lo:P, t0 * TC:(t0 + 1) * TC])
                    t0 += 1
                t1 = r1 // P                    # full windows t0..t1-1
                if t1 > t0:
                    off = t0 * P - r0
                    nc.sync.dma_start(
                        out=bounce[j].ap()[off:off + (t1 - t0) * P, :]
                        .rearrange("(t p) c -> p t c", p=P),
                        in_=hw_stage[:, t0 * TC:t1 * TC]
                        .rearrange("p (t c) -> p t c", c=TC))
                if r1 % P:                      # tail: partial window t1
                    hi = r1 % P
                    nc.sync.dma_start(
                        out=bounce[j].ap()[t1 * P - r0:t1 * P - r0 + hi, :],
                        in_=hw_stage[0:hi, t1 * TC:(t1 + 1) * TC])

            def gather_quarter(j, tset):
                """AllGather the bounced quarter into the replicated table."""
                nc.gpsimd.collective_compute(
                    "AllGather", mybir.AluOpType.bypass,
                    replica_groups=[list(range(C))],
                    ins=[bounce[j].ap().opt()],
                    outs=[tables[tset][j].ap().opt()])

            # conv-0 tables from x
            conv0_fill = [0] * cfg.NQ
            for t in range(NT):
                gemm_tile(0, t)
                for jq in quarters_of_window[t]:
                    conv0_fill[jq] += 1
                    if conv0_fill[jq] == qwin_count[jq]:
                        bounce_quarter(jq, 0)
                        gather_quarter(jq, 0)

            pool_state = dict(psP=None, closed=0)

            def finalize_tile(layer, w, acc_tile, wi):
                """Window w's PSUM accumulation complete: self-loop add,
                epilogue, then feed forward (next GEMM, or pooling)."""
                psum_w = acc_tile[:, wi * F:(wi + 1) * F]
                o_t = o_shard[:, w * F:(w + 1) * F]
                hw_w = hw_stage[:, w * TC:w * TC + F]
                # self-loop msg after epilogue scale: dinv_d^2*(hW)[d]
                # = dinv_d * T[d] where hw_stage holds T = dinv*(hW)
                u = tmp.tile([P, F], f32, tag="selfu")
                nc.scalar.mul(u[:], hw_w, dinvt_sb[:, w:w + 1])
                # v = dinv_d * psum (epilogue scale of aggregated sum)
                v = tmp.tile([P, F], f32, tag="epv")
                nc.scalar.mul(v[:], psum_w, dinvt_sb[:, w:w + 1])
                # o = u + v + bias (+relu for layer 0)
                tt = tmp.tile([P, F], f32, tag="ep")
                nc.vector.tensor_tensor(tt[:], u[:], v[:],
                                        op=mybir.AluOpType.add)
                if layer == 0:
                    nc.vector.tensor_tensor(
                        tt[:], tt[:], bias_sb[:, layer * F:(layer + 1) * F],
                        op=mybir.AluOpType.add)
                    nc.scalar.activation(o_t, tt[:],
                                         mybir.ActivationFunctionType.Relu)
                else:
                    nc.vector.tensor_tensor(
                        o_t, tt[:], bias_sb[:, layer * F:(layer + 1) * F],
                        op=mybir.AluOpType.add)
                if layer < 2:
                    gemm_tile(layer + 1, w)
                    for jq in quarters_of_window[w]:
                        quarter_fill[jq] += 1
                        if quarter_fill[jq] == qwin_count[jq]:
                            # bounce now; AllGather after ship_delay calls
                            bounce_quarter(jq, (layer + 1) % 2)
                            pending_ships.append(
                                [cfg.ship_delay, jq, (layer + 1) % 2])
                else:
                    # tensor_tensor is_equal: the tensor_scalar PTR path
                    # costs ~3.9us/op on DVE vs ~0.1us for TT broadcast
                    Gt = spool.tile([P, G], f32, tag="Gt")
                    nc.vector.tensor_tensor(
                        Gt[:],
                        batchloc_sb[:, w:w + 1].rearrange(
                            "p (o g) -> p o g", o=1).to_broadcast([P, 1, G]),
                        iota_sb[:, :G].rearrange("p (o g) -> p o g", o=1),
                        op=mybir.AluOpType.is_equal)
                    if pool_state["psP"] is None:
                        pool_state["psP"] = ps_mm.tile(
                            [G, F], f32, tag="psG", name="psP")
                    pool_state["closed"] += 1
                    nc.tensor.matmul(
                        pool_state["psP"][:], lhsT=Gt[:], rhs=o_t,
                        start=(pool_state["closed"] == 1),
                        stop=(pool_state["closed"] == NT))

            pending_ships = []

            def tick_ships(force=False):
                for ent in pending_ships:
                    ent[0] -= 1
                while pending_ships and (force or pending_ships[0][0] <= 0):
                    _, jq, tset = pending_ships.pop(0)
                    gather_quarter(jq, tset)

            for layer in range(3):
                quarter_fill = [0] * cfg.NQ
                # persistent PSUM accumulator per superblock (ping-pong)
                acc_of_sb = {}
                # windows closed during call i finalize AFTER call i+1's
                # gather+S-build are issued: keeps the next call's DVE work
                # ahead of the epilogue in the in-order vector queue
                deferred = []

                for ci, call in enumerate(sched["calls"]):
                    tick_ships()
                    n, gs0, qq = call["n"], call["gs0"], call["q"]
                    gt = gbuf.tile([P, cfg.GCH * TC], SDT, tag="gt")
                    idxs_ap = gidx_sb[:, 8 * gs0:8 * (gs0 + n)]
                    nc.gpsimd.dma_gather(
                        gt[:].rearrange("p (n c) -> p n c", c=TC)[:, :n, :],
                        tables[layer % 2][qq][:, :],
                        idxs_ap,
                        n * P, n * P, TC,
                        single_packet=cfg.single_packet,
                        queue_num=ci % cfg.swdge_queues)
                    # one-hot selection matrices for the whole call, one DVE
                    # op in 2x mode (all operands 2-byte packed innermost)
                    S_b = spool.tile([P, cfg.GCH * P], SDT, tag="S")
                    dl_ap = dstloc_sb[:, 2 * gs0:2 * (gs0 + n)] \
                        .rearrange("p (n one two) -> p n one two",
                                   one=1, two=2) \
                        .to_broadcast([P, n, P // 2, 2])
                    io_ap = iota_sdt_sb[:].rearrange(
                        "p (o a b) -> p o a b", o=1, b=2) \
                        .to_broadcast([P, n, P // 2, 2])
                    nc.vector.tensor_tensor(
                        S_b[:, :n * P].rearrange("p (n a b) -> p n a b",
                                                 a=P // 2, b=2),
                        dl_ap, io_ap,
                        op=mybir.AluOpType.is_equal)
                    closing = []
                    for j in range(n):
                        gs = gs0 + j
                        w = int(sub_w[gs])
                        sb = int(win_sb[w])
                        wi = int(win_pos[w])
                        if sb not in acc_of_sb:
                            acc_of_sb[sb] = ps_acc.tile(
                                [P, SBW * F], f32, tag="acc", name="acc")
                            # hardware mis-accumulates interleaved start/stop
                            # groups; zero the region and accumulate with
                            # start=False throughout instead
                            nc.vector.memset(acc_of_sb[sb][:], 0.0)
                        acc_tile = acc_of_sb[sb]
                        nc.tensor.matmul(
                            acc_tile[:, wi * F:(wi + 1) * F],
                            lhsT=S_b[:, j * P:(j + 1) * P],
                            rhs=gt[:, j * TC:j * TC + F],
                            start=False, stop=True, skip_group_check=True)
                        if sub_last[gs]:
                            closing.append((layer, w, acc_tile, wi))
                    # flush PREVIOUS call's finalizes after this call's
                    # matmuls (so gt frees ASAP for the gather pipeline),
                    # then queue this call's closings for the next flush
                    for args in deferred:
                        finalize_tile(*args)
                    deferred = closing

                for args in deferred:
                    finalize_tile(*args)
                # don't force-flush ships at the layer boundary: the last
                # quarter's AllGather (whose bounce waits on the final
                # windows' epilogue+GEMM chain) would stall the gpsimd queue
                # ahead of the next layer's gathers; let it tick out during
                # the next layer's early calls instead
            tick_ships(force=True)

            # ---- pooled sums across cores
            sums = tmp.tile([G, F], f32, tag="sums")
            nc.scalar.copy(sums[:], pool_state["psP"][:])
            nc.sync.dma_start(out=pool_in[:, :], in_=sums[:])
            nc.gpsimd.collective_compute(
                "AllReduce", mybir.AluOpType.add,
                replica_groups=[list(range(C))],
                ins=[pool_in.ap().opt()],
                outs=[pool_out.ap().opt()])
            sums2 = tmp.tile([G, F], f32, tag="sums")
            nc.sync.dma_start(out=sums2[:], in_=pool_out[:, :])
            res = tmp.tile([G, cfg.OUT], f32, tag="res")
            nc.vector.tensor_scalar_mul(res[:], sums2[:, :cfg.OUT], invcnt_sb[:])
            nc.sync.dma_start(out=out_dram[:, :], in_=res[:])

    return nc


# --------------------------------------------------------------------------
# Entry point
# --------------------------------------------------------------------------

def _install_trace_hooks():
    """The agent image's antenv lacks axon_hooks; reconstruct it so
    run_bass_kernel_spmd(trace=True) can NTFF-profile via ctypes, and stub
    the S3 artifact upload."""
    import types
    import antenv
    if "antenv.axon_hooks" not in sys.modules:
        mod = types.ModuleType("antenv.axon_hooks")
        mod._hook = None
        def _set(h):
            mod._hook = h
        def _get():
            return mod._hook
        mod.set_axon_ntff_profile_hook = _set
        mod.get_axon_ntff_profile_hook = _get
        sys.modules["antenv.axon_hooks"] = mod
        antenv.axon_hooks = mod
    hooks = sys.modules["antenv.axon_hooks"]
    if hooks.get_axon_ntff_profile_hook() is None:
        if "/root/.axon_site" not in sys.path:
            sys.path.insert(0, "/root/.axon_site")
        from trn_agent_boot.trn_boot import _ntff_profile_via_ctypes
        hooks.set_axon_ntff_profile_hook(
            _ntff_profile_via_ctypes("/opt/axon/libaxon_pjrt.so"))
    import concourse.bass_utils as bu
    bu.upload_artifacts = lambda tmpdir: tmpdir


def kernel(x, edge_index, batch, num_graphs, W1, b1, W2, b2, W3, b3,
           _trace=False, _cfg=None):
    cfg = _cfg or FULL
    assert int(num_graphs) == cfg.G
    sched, in_maps = host_prep(x, edge_index, batch, W1, b1, W2, b2, W3, b3, cfg)
    nc = build_program(sched, cfg)
    nc.finalize()

    if _trace:
        _install_trace_hooks()
    from concourse.bass_utils import run_bass_kernel_spmd
    res = run_bass_kernel_spmd(nc, in_maps, core_ids=list(range(cfg.C)),
                               trace=_trace)
    out = np.asarray(res.results[0]["out"], dtype=np.float32)
    if _trace:
        return out, res.exec_time_ns
    return out
